# revision 1
# baseline (speedup 1.0000x reference)
"""Trainium2 Bass kernel for strided-conv-as-linear (nn_ConvNd_60851096649851).

Computation (see reference): x [B,1024,1024] f32, weight [16,256] f32.
16x16 windows at stride 8 -> 127x127 patches; per patch y = W @ flat(window)
(16 outputs), reshaped to a 4x4 tile of the [B,508,508] output.

Strategy: data-parallel over batch (4 images per core, 8 cores).
Per image: 9 overlapping 128-row window tiles (stride 120 rows, last tile
anchored at H-128). Rows live on SBUF partitions (natural layout). For each
window tile, out[(i_l,o), j] = sum_kw Wband_kw[row, (i_l,o)]^T @ x[row, 8j+kw]
where Wband_kw is the banded weight (nonzero at row = 8*i_l + kh): 16
accumulating float32r matmuls with stride-8 column APs, K=128, M=128
(i_l in [0,8) x 16 outputs per half; two halves cover 15 patches/tile).
Three tiles are batched in the moving dim (N = 3*127 = 381 >= 256) so
float32r streams at 1 cycle/column. PSUM holds the final outputs; DVE
copies to SBUF; contiguous DMA out. Host prepares banded weights and
unscrambles the device output layout into [B,508,508].
"""

import os
import sys

sys.path.insert(0, "/opt/trn_rl_repo")
os.environ.setdefault("JAX_PLATFORMS", "cpu")

import numpy as np

import concourse.bass as bass  # noqa: F401
import concourse.tile as tile
from concourse import bacc, mybir
from concourse.bass_utils import run_bass_kernel_spmd

N_CORES = 8
KH = KW = 16
STRIDE = 8
D0 = D1 = 4  # per-patch output tile
OC = 16  # outputs per patch = D0*D1
PATCHES_PER_TILE = 15  # full patches in a 128-row window
GROUP = 3  # window tiles batched per matmul (moving dim)

_MM_DTYPE = mybir.dt.float32r


def _tile_starts(H):
    """Start rows of 128-row window tiles covering all patch rows."""
    nH = (H - KH) // STRIDE + 1
    starts = []
    i = 0
    while i < nH:
        s = min(STRIDE * i, H - 128)
        starts.append(s)
        if s == H - 128:
            break
        i += PATCHES_PER_TILE
    return starts, nH


def build_wband(weight):
    """Banded weights: [128, KW*2*128] f32.

    wb[p, kw, h, m] = W[o, kh*16+kw] where p = 8*(i_l+8h)+kh, m = i_l*16+o,
    i_l in [0,8), valid patch slots i_l+8h in [0,15).
    """
    W4 = np.asarray(weight, np.float32).reshape(OC, KH, KW)
    wb = np.zeros((128, KW, 2, 128), np.float32)
    for h in range(2):
        for il in range(8):
            ig = il + 8 * h
            if ig >= PATCHES_PER_TILE:
                continue
            for kh in range(KH):
                p = 8 * ig + kh
                wb[p, :, h, il * OC : (il + 1) * OC] = W4[:, kh, :].T
    return np.ascontiguousarray(wb.reshape(128, KW * 2 * 128))


def build_nc(n_img, H, W):
    """Build the per-core Bass program. Returns compiled nc."""
    starts, nH = _tile_starts(H)
    nW = (W - KW) // STRIDE + 1
    n_tiles = len(starts)
    assert n_tiles % GROUP == 0, (n_tiles, GROUP)
    n_groups = n_tiles // GROUP
    nWp = ((nW + 3) // 4) * 4  # fp32r needs even moving size; pad j
    NF = GROUP * nWp  # moving free size per matmul
    WS = W + STRIDE  # padded slot width so the padded-j column stays in bounds

    nc = bacc.Bacc(
        "TRN2", target_bir_lowering=False, debug=False, num_devices=N_CORES
    )
    f32 = mybir.dt.float32
    x_d = nc.dram_tensor("x", [n_img, H, W], f32, kind="ExternalInput").ap()
    wb_d = nc.dram_tensor(
        "wb", [128, KW * 2 * 128], f32, kind="ExternalInput"
    ).ap()
    out_d = nc.dram_tensor(
        "out", [n_img, n_groups, 2, 128, NF], f32, kind="ExternalOutput"
    ).ap()

    with tile.TileContext(nc) as tc:
        with (
            tc.tile_pool(name="wbp", bufs=1) as wbp,
            tc.tile_pool(name="xp", bufs=2) as xp,
            tc.tile_pool(name="psp", bufs=2 * n_groups, space="PSUM") as psp,
            tc.tile_pool(name="op", bufs=4) as op,
        ):
            wb_sb = wbp.tile([128, KW * 2 * 128], _MM_DTYPE)
            nc.gpsimd.dma_start(wb_sb[:], wb_d[:])

            for b in range(n_img):
                xg = xp.tile([128, n_tiles * WS], _MM_DTYPE)
                for t, s in enumerate(starts):
                    nc.gpsimd.dma_start(
                        xg[:, t * WS : t * WS + W], x_d[b, s : s + 128, :]
                    )
                    nc.gpsimd.dma_start(
                        xg[:, t * WS + W : (t + 1) * WS],
                        x_d[b, s : s + 128, 0:STRIDE],
                    )
                xg3 = xg.rearrange("p (t w) -> p t w", t=n_tiles)

                ps = [
                    [
                        psp.tile([128, NF], f32, name=f"ps_{b}_{g}_{h}", tag="ps")
                        for h in range(2)
                    ]
                    for g in range(n_groups)
                ]
                for kw in range(KW):
                    for h in range(2):
                        lhsT = wb_sb[
                            :, (kw * 2 + h) * 128 : (kw * 2 + h) * 128 + 128
                        ]
                        for g in range(n_groups):
                            rhs = xg3[
                                :,
                                g * GROUP : (g + 1) * GROUP,
                                kw : kw + STRIDE * (nWp - 1) + 1 : STRIDE,
                            ]
                            nc.tensor.matmul(
                                ps[g][h][:],
                                lhsT,
                                rhs,
                                start=(kw == 0),
                                stop=(kw == KW - 1),
                            )
                for g in range(n_groups):
                    for h in range(2):
                        ob = op.tile([128, NF], f32, name="ob")
                        nc.vector.tensor_copy(ob[:], ps[g][h][:])
                        nc.sync.dma_start(out_d[b, g, h], ob[:])
    nc.compile()
    return nc, starts, nH, nW, n_groups


def unscramble(dev_out, starts, nH, nW, n_img):
    """dev_out [n_img, n_groups, 2, 128, GROUP*nW] -> [n_img, nH*4, nW*4]."""
    n_groups = dev_out.shape[1]
    nWp = ((nW + 3) // 4) * 4
    dev = dev_out.reshape(n_img, n_groups, 2, 8, D0, D1, GROUP, nWp)[..., :nW]
    out5 = np.empty((n_img, nH, D0, nW, D1), np.float32)
    filled = np.zeros(nH, bool)
    for g in range(n_groups):
        for t in range(GROUP):
            tau = g * GROUP + t
            i0 = starts[tau] // STRIDE
            for h in range(2):
                for il in range(8):
                    ig = il + 8 * h
                    i = i0 + ig
                    if ig >= PATCHES_PER_TILE or i >= nH or filled[i]:
                        continue
                    # dev[b, g, h, il, d0, d1, t, j] -> out5[b, i, d0, j, d1]
                    out5[:, i] = dev[:, g, h, il, :, :, t, :].transpose(0, 1, 3, 2)
                    filled[i] = True
    assert filled.all()
    return out5.reshape(n_img, nH * D0, nW * D1)


def _pjrt_timed_run(nc, in_maps, n_iters=6):
    """Clone of bass2jax.run_bass_via_pjrt's multi-core path, without
    donation, with device-resident inputs, timing each execution.
    Returns (results_per_core, [wall_seconds per iter])."""
    import time

    import jax
    from jax.sharding import Mesh, PartitionSpec
    from jax.experimental.shard_map import shard_map

    from concourse import bass2jax, mybir as _mb
    from concourse.bass2jax import _bass_exec_p, partition_id_tensor

    bass2jax.install_neuronx_cc_hook()
    n_cores = len(in_maps)
    partition_name = nc.partition_id_tensor.name if nc.partition_id_tensor else None

    in_names, out_names, out_avals = [], [], []
    zero_outs = []
    for alloc in nc.m.functions[0].allocations:
        if not isinstance(alloc, _mb.MemoryLocationSet):
            continue
        name = alloc.memorylocations[0].name
        if alloc.kind == "ExternalInput":
            if name != partition_name:
                in_names.append(name)
        elif alloc.kind == "ExternalOutput":
            shape = tuple(alloc.tensor_shape)
            dtype = _mb.dt.np(alloc.dtype)
            out_names.append(name)
            out_avals.append(jax.core.ShapedArray(shape, dtype))
            zero_outs.append(np.zeros(shape, dtype))
    n_params = len(in_names)
    in_names_all = in_names + out_names
    if partition_name is not None:
        in_names_all.append(partition_name)

    def _body(*args):
        operands = list(args)
        if partition_name is not None:
            operands.append(partition_id_tensor())
        outs = _bass_exec_p.bind(
            *operands,
            out_avals=tuple(out_avals),
            in_names=tuple(in_names_all),
            out_names=tuple(out_names),
            lowering_input_output_aliases=(),
            sim_require_finite=True,
            sim_require_nnan=True,
            nc=nc,
        )
        return tuple(outs)

    devices = jax.devices()[:n_cores]
    mesh = Mesh(np.asarray(devices), ("core",))
    in_specs = (PartitionSpec("core"),) * (n_params + len(out_names))
    out_specs = (PartitionSpec("core"),) * len(out_names)
    sharded = jax.jit(
        shard_map(_body, mesh=mesh, in_specs=in_specs, out_specs=out_specs,
                  check_rep=False),
        keep_unused=True,
    )
    concat_in = [
        np.concatenate([np.asarray(in_maps[c][n]) for c in range(n_cores)], axis=0)
        for n in in_names
    ]
    concat_zeros = [
        np.zeros((n_cores * z.shape[0], *z.shape[1:]), z.dtype) for z in zero_outs
    ]
    from jax.sharding import NamedSharding

    dev_in = [
        jax.device_put(a, NamedSharding(mesh, PartitionSpec("core")))
        for a in concat_in + concat_zeros
    ]
    out_arrs = sharded(*dev_in)  # warmup + compile
    jax.block_until_ready(out_arrs)
    times = []
    for _ in range(n_iters):
        t0 = time.perf_counter()
        out_arrs = sharded(*dev_in)
        jax.block_until_ready(out_arrs)
        times.append(time.perf_counter() - t0)
    results = [
        {
            n: np.asarray(out_arrs[i]).reshape(n_cores, *out_avals[i].shape)[c]
            for i, n in enumerate(out_names)
        }
        for c in range(n_cores)
    ]
    return results, times


_CACHE = {}


def _get_nc(n_img, H, W):
    key = (n_img, H, W)
    if key not in _CACHE:
        _CACHE[key] = build_nc(n_img, H, W)
    return _CACHE[key]


def kernel(x, weight, _timed=False):
    x = np.asarray(x, np.float32)
    weight = np.asarray(weight, np.float32)
    B, H, W = x.shape
    assert B % N_CORES == 0
    n_img = B // N_CORES
    nc, starts, nH, nW, n_groups = _get_nc(n_img, H, W)
    wb = build_wband(weight)
    in_maps = [
        {"x": np.ascontiguousarray(x[c * n_img : (c + 1) * n_img]), "wb": wb}
        for c in range(N_CORES)
    ]
    if _timed:
        results, times = _pjrt_timed_run(nc, in_maps)
    else:
        results = run_bass_kernel_spmd(
            nc, in_maps, core_ids=list(range(N_CORES))
        ).results
        times = None
    shards = [
        unscramble(results[c]["out"], starts, nH, nW, n_img)
        for c in range(N_CORES)
    ]
    full = np.concatenate(shards, axis=0)
    if _timed:
        return full, times
    return full



# revision 2
# speedup vs baseline: 245.3532x; 245.3532x over previous
"""Trainium2 Bass kernel for strided-conv-as-linear (nn_ConvNd_60851096649851).

Computation (see reference): x [B,1024,1024] f32, weight [16,256] f32.
16x16 windows at stride 8 -> 127x127 patches; per patch y = W @ flat(window)
(16 outputs), reshaped to a 4x4 tile of the [B,508,508] output.

Strategy: data-parallel over batch (4 images per core, 8 cores).
Per image: 9 overlapping 128-row window tiles (stride 120 rows, last tile
anchored at H-128). Rows live on SBUF partitions (natural layout). For each
window tile, out[(i_l,o), j] = sum_kw Wband_kw[row, (i_l,o)]^T @ x[row, 8j+kw]
where Wband_kw is the banded weight (nonzero at row = 8*i_l + kh): 16
accumulating bf16 matmuls with stride-8 column APs, K=128, M=128
(i_l in [0,8) x 16 outputs per half; two halves cover 15 patches/tile).
Three tiles are batched in the moving dim (N = 3*128 = 384) so the PE
streams at 1 cycle/column in bf16. PSUM holds the f32 outputs; DVE
copies to SBUF; contiguous DMA out. Host casts x to bf16 (and pads the
row width by 8 so the padded j column stays in bounds), prepares banded
bf16 weights, and unscrambles the device output layout into [B,508,508].
"""

import os
import sys

sys.path.insert(0, "/opt/trn_rl_repo")
os.environ.setdefault("JAX_PLATFORMS", "cpu")

import numpy as np

import concourse.bass as bass  # noqa: F401
import concourse.tile as tile
from concourse import bacc, mybir
from concourse.bass_utils import run_bass_kernel_spmd

N_CORES = 8
KH = KW = 16
STRIDE = 8
D0 = D1 = 4  # per-patch output tile
OC = 16  # outputs per patch = D0*D1
PATCHES_PER_TILE = 15  # full patches in a 128-row window
GROUP = 3  # window tiles batched per matmul (moving dim)

_MM_DTYPE = mybir.dt.bfloat16
_BF16 = mybir.dt.np(mybir.dt.bfloat16)


def _tile_starts(H):
    """Start rows of 128-row window tiles covering all patch rows."""
    nH = (H - KH) // STRIDE + 1
    starts = []
    i = 0
    while i < nH:
        s = min(STRIDE * i, H - 128)
        starts.append(s)
        if s == H - 128:
            break
        i += PATCHES_PER_TILE
    return starts, nH


def build_wband(weight):
    """Banded weights: [128, KW*2*128] bf16.

    wb[p, kw, h, m] = W[o, kh*16+kw] where p = 8*(i_l+8h)+kh, m = i_l*16+o,
    i_l in [0,8), valid patch slots i_l+8h in [0,15).
    """
    W4 = np.asarray(weight, np.float32).reshape(OC, KH, KW)
    wb = np.zeros((128, KW, 2, 128), np.float32)
    for h in range(2):
        for il in range(8):
            ig = il + 8 * h
            if ig >= PATCHES_PER_TILE:
                continue
            for kh in range(KH):
                p = 8 * ig + kh
                wb[p, :, h, il * OC : (il + 1) * OC] = W4[:, kh, :].T
    return np.ascontiguousarray(wb.reshape(128, KW * 2 * 128)).astype(_BF16)


def build_nc(n_img, H, W):
    """Build the per-core Bass program. Returns compiled nc."""
    starts, nH = _tile_starts(H)
    nW = (W - KW) // STRIDE + 1
    n_tiles = len(starts)
    assert n_tiles % GROUP == 0, (n_tiles, GROUP)
    n_groups = n_tiles // GROUP
    nWp = ((nW + 3) // 4) * 4  # pad j to multiple of 4
    NF = GROUP * nWp  # moving free size per matmul
    WS = W + STRIDE  # host-padded row width so the padded-j column is in bounds

    nc = bacc.Bacc(
        "TRN2", target_bir_lowering=False, debug=False, num_devices=N_CORES
    )
    f32 = mybir.dt.float32
    x_d = nc.dram_tensor("x", [n_img, H, WS], _MM_DTYPE, kind="ExternalInput").ap()
    wb_d = nc.dram_tensor(
        "wb", [128, KW * 2 * 128], _MM_DTYPE, kind="ExternalInput"
    ).ap()
    out_d = nc.dram_tensor(
        "out", [n_img, n_groups, 2, 128, NF], f32, kind="ExternalOutput"
    ).ap()

    with tile.TileContext(nc) as tc:
        with (
            tc.tile_pool(name="wbp", bufs=1) as wbp,
            tc.tile_pool(name="xp", bufs=2) as xp,
            tc.tile_pool(name="psp", bufs=2 * n_groups, space="PSUM") as psp,
            tc.tile_pool(name="op", bufs=4) as op,
        ):
            wb_sb = wbp.tile([128, KW * 2 * 128], _MM_DTYPE)
            nc.gpsimd.dma_start(wb_sb[:], wb_d[:])

            for b in range(n_img):
                xg = xp.tile([128, n_tiles * WS], _MM_DTYPE)
                for t, s in enumerate(starts):
                    nc.gpsimd.dma_start(
                        xg[:, t * WS : (t + 1) * WS], x_d[b, s : s + 128, :]
                    )
                xg3 = xg.rearrange("p (t w) -> p t w", t=n_tiles)

                ps = [
                    [
                        psp.tile([128, NF], f32, name=f"ps_{b}_{g}_{h}", tag="ps")
                        for h in range(2)
                    ]
                    for g in range(n_groups)
                ]
                for kw in range(KW):
                    for h in range(2):
                        lhsT = wb_sb[
                            :, (kw * 2 + h) * 128 : (kw * 2 + h) * 128 + 128
                        ]
                        for g in range(n_groups):
                            rhs = xg3[
                                :,
                                g * GROUP : (g + 1) * GROUP,
                                kw : kw + STRIDE * (nWp - 1) + 1 : STRIDE,
                            ]
                            nc.tensor.matmul(
                                ps[g][h][:],
                                lhsT,
                                rhs,
                                start=(kw == 0),
                                stop=(kw == KW - 1),
                            )
                for g in range(n_groups):
                    for h in range(2):
                        ob = op.tile([128, NF], f32, name="ob")
                        nc.vector.tensor_copy(ob[:], ps[g][h][:])
                        nc.sync.dma_start(out_d[b, g, h], ob[:])
    nc.compile()
    return nc, starts, nH, nW, n_groups


def unscramble(dev_out, starts, nH, nW, n_img):
    """dev_out [n_img, n_groups, 2, 128, GROUP*nWp] -> [n_img, nH*4, nW*4]."""
    n_groups = dev_out.shape[1]
    nWp = ((nW + 3) // 4) * 4
    dev = dev_out.reshape(n_img, n_groups, 2, 8, D0, D1, GROUP, nWp)[..., :nW]
    out5 = np.empty((n_img, nH, D0, nW, D1), np.float32)
    filled = np.zeros(nH, bool)
    for g in range(n_groups):
        for t in range(GROUP):
            tau = g * GROUP + t
            i0 = starts[tau] // STRIDE
            for h in range(2):
                for il in range(8):
                    ig = il + 8 * h
                    i = i0 + ig
                    if ig >= PATCHES_PER_TILE or i >= nH or filled[i]:
                        continue
                    # dev[b, g, h, il, d0, d1, t, j] -> out5[b, i, d0, j, d1]
                    out5[:, i] = dev[:, g, h, il, :, :, t, :].transpose(0, 1, 3, 2)
                    filled[i] = True
    assert filled.all()
    return out5.reshape(n_img, nH * D0, nW * D1)


def prep_x(x, n_img):
    """Cast to bf16 and pad width by STRIDE; split per core."""
    B, H, W = x.shape
    xb = np.zeros((B, H, W + STRIDE), _BF16)
    xb[:, :, :W] = x.astype(_BF16)
    return [
        np.ascontiguousarray(xb[c * n_img : (c + 1) * n_img])
        for c in range(N_CORES)
    ]


_CACHE = {}


def _get_nc(n_img, H, W):
    key = (n_img, H, W)
    if key not in _CACHE:
        _CACHE[key] = build_nc(n_img, H, W)
    return _CACHE[key]


def kernel(x, weight):
    x = np.asarray(x, np.float32)
    weight = np.asarray(weight, np.float32)
    B, H, W = x.shape
    assert B % N_CORES == 0
    n_img = B // N_CORES
    nc, starts, nH, nW, n_groups = _get_nc(n_img, H, W)
    wb = build_wband(weight)
    x_shards = prep_x(x, n_img)
    in_maps = [{"x": x_shards[c], "wb": wb} for c in range(N_CORES)]
    results = run_bass_kernel_spmd(
        nc, in_maps, core_ids=list(range(N_CORES))
    ).results
    shards = [
        unscramble(results[c]["out"], starts, nH, nW, n_img)
        for c in range(N_CORES)
    ]
    return np.concatenate(shards, axis=0)


# revision 6
# speedup vs baseline: 717.0402x; 2.9225x over previous
"""Trainium2 Bass kernel for strided-conv-as-linear (nn_ConvNd_60851096649851).

Computation (see reference): x [B,1024,1024] f32, weight [16,256] f32.
16x16 windows at stride 8 -> 127x127 patches; per patch y = W @ flat(window)
(16 outputs), reshaped to a 4x4 tile of the [B,508,508] output.

Strategy: data-parallel over batch (4 images per core, 8 cores).
Per image: 9 overlapping 128-row window tiles (stride 120 rows, last tile
anchored at H-128). Rows live on SBUF partitions (natural layout). For each
window tile, out[(i_l,o), j] = sum_kw Wband_kw[row, (i_l,o)]^T @ x[row, 8j+kw]
where Wband_kw is the banded weight (nonzero at row = 8*i_l + kh): 16
accumulating bf16 matmuls with stride-8 column APs, K=128, M=128
(i_l in [0,8) x 16 outputs per half; two halves cover 15 patches/tile).
Three tiles are batched in the moving dim (N = 3*128 = 384) so the PE
streams at 1 cycle/column in bf16. PSUM holds the f32 outputs; DVE
copies to SBUF; contiguous DMA out. Host casts x to bf16 (and pads the
row width by 8 so the padded j column stays in bounds), prepares banded
bf16 weights, and unscrambles the device output layout into [B,508,508].
"""

import os
import sys

sys.path.insert(0, "/opt/trn_rl_repo")
os.environ.setdefault("JAX_PLATFORMS", "cpu")

import numpy as np

import concourse.bass as bass  # noqa: F401
import concourse.tile as tile
from concourse import bacc, mybir
from concourse.bass_utils import run_bass_kernel_spmd

N_CORES = 8
KH = KW = 16
STRIDE = 8
D0 = D1 = 4  # per-patch output tile
OC = 16  # outputs per patch = D0*D1
PATCHES_PER_TILE = 15  # full patches in a 128-row window
GROUP = 3  # window tiles batched per matmul (moving dim)

_MM_DTYPE = mybir.dt.bfloat16
_BF16 = mybir.dt.np(mybir.dt.bfloat16)


def _tile_starts(H):
    """Start rows of 128-row window tiles covering all patch rows."""
    nH = (H - KH) // STRIDE + 1
    starts = []
    i = 0
    while i < nH:
        s = min(STRIDE * i, H - 128)
        starts.append(s)
        if s == H - 128:
            break
        i += PATCHES_PER_TILE
    return starts, nH


def build_wband(weight):
    """Banded weights: [128, KW*2*128] bf16.

    wb[p, kw, h, m] = W[o, kh*16+kw] where p = 8*(i_l+8h)+kh, m = i_l*16+o,
    i_l in [0,8), valid patch slots i_l+8h in [0,15).
    """
    W4 = np.asarray(weight, np.float32).reshape(OC, KH, KW)
    wb = np.zeros((128, KW, 2, 128), np.float32)
    for h in range(2):
        for il in range(8):
            ig = il + 8 * h
            if ig >= PATCHES_PER_TILE:
                continue
            for kh in range(KH):
                p = 8 * ig + kh
                wb[p, :, h, il * OC : (il + 1) * OC] = W4[:, kh, :].T
    return np.ascontiguousarray(wb.reshape(128, KW * 2 * 128)).astype(_BF16)


def build_nc(n_img, H, W):
    """Build the per-core Bass program. Returns compiled nc."""
    starts, nH = _tile_starts(H)
    nW = (W - KW) // STRIDE + 1
    n_tiles = len(starts)
    assert n_tiles % GROUP == 0, (n_tiles, GROUP)
    n_groups = n_tiles // GROUP
    nWp = ((nW + 3) // 4) * 4  # pad j to multiple of 4
    NF = GROUP * nWp  # moving free size per matmul
    NM = W // STRIDE + 1  # phase-deinterleaved positions per row (129)
    WS = STRIDE * NM  # host-padded row width (phase-major layout)

    nc = bacc.Bacc(
        "TRN2", target_bir_lowering=False, debug=False, num_devices=N_CORES
    )
    f32 = mybir.dt.float32
    x_d = nc.dram_tensor("x", [n_img, H, WS], _MM_DTYPE, kind="ExternalInput").ap()
    wb_d = nc.dram_tensor(
        "wb", [128, KW * 2 * 128], _MM_DTYPE, kind="ExternalInput"
    ).ap()
    out_d = nc.dram_tensor(
        "out", [n_img, n_groups, 2, 128, NF], f32, kind="ExternalOutput"
    ).ap()

    with tile.TileContext(nc) as tc:
        with (
            tc.tile_pool(name="wbp", bufs=1) as wbp,
            tc.tile_pool(name="xp", bufs=2) as xp,
            tc.tile_pool(name="psp", bufs=2 * n_groups, space="PSUM") as psp,
            tc.tile_pool(name="op", bufs=4) as op,
        ):
            wb_sb = wbp.tile([128, KW * 2 * 128], _MM_DTYPE)
            nc.gpsimd.dma_start(wb_sb[:], wb_d[:])

            for b in range(n_img):
                xg = xp.tile([128, n_tiles * WS], _MM_DTYPE)
                for t, s in enumerate(starts):
                    nc.gpsimd.dma_start(
                        xg[:, t * WS : (t + 1) * WS], x_d[b, s : s + 128, :]
                    )
                xg4 = xg.rearrange(
                    "p (t f m) -> p t f m", t=n_tiles, f=STRIDE, m=NM
                )

                ps = [
                    [
                        psp.tile([128, NF], f32, name=f"ps_{b}_{g}_{h}", tag="ps")
                        for h in range(2)
                    ]
                    for g in range(n_groups)
                ]
                for kw in range(KW):
                    for h in range(2):
                        lhsT = wb_sb[
                            :, (kw * 2 + h) * 128 : (kw * 2 + h) * 128 + 128
                        ]
                        m0 = 0 if kw < STRIDE else 1
                        for g in range(n_groups):
                            rhs = xg4[
                                :,
                                g * GROUP : (g + 1) * GROUP,
                                kw % STRIDE,
                                m0 : m0 + nWp,
                            ]
                            nc.tensor.matmul(
                                ps[g][h][:],
                                lhsT,
                                rhs,
                                start=(kw == 0),
                                stop=(kw == KW - 1),
                            )
                for g in range(n_groups):
                    for h in range(2):
                        ob = op.tile([128, NF], f32, name="ob")
                        nc.vector.tensor_copy(ob[:], ps[g][h][:])
                        nc.sync.dma_start(out_d[b, g, h], ob[:])
    nc.compile()
    return nc, starts, nH, nW, n_groups


def unscramble(dev_out, starts, nH, nW, n_img):
    """dev_out [n_img, n_groups, 2, 128, GROUP*nWp] -> [n_img, nH*4, nW*4]."""
    n_groups = dev_out.shape[1]
    nWp = ((nW + 3) // 4) * 4
    dev = dev_out.reshape(n_img, n_groups, 2, 8, D0, D1, GROUP, nWp)[..., :nW]
    out5 = np.empty((n_img, nH, D0, nW, D1), np.float32)
    filled = np.zeros(nH, bool)
    for g in range(n_groups):
        for t in range(GROUP):
            tau = g * GROUP + t
            i0 = starts[tau] // STRIDE
            for h in range(2):
                for il in range(8):
                    ig = il + 8 * h
                    i = i0 + ig
                    if ig >= PATCHES_PER_TILE or i >= nH or filled[i]:
                        continue
                    # dev[b, g, h, il, d0, d1, t, j] -> out5[b, i, d0, j, d1]
                    out5[:, i] = dev[:, g, h, il, :, :, t, :].transpose(0, 1, 3, 2)
                    filled[i] = True
    assert filled.all()
    return out5.reshape(n_img, nH * D0, nW * D1)


def prep_x(x, n_img):
    """Cast to bf16, pad width by STRIDE, and phase-deinterleave each row:
    xd[b, r, phi, m] = x_pad[b, r, STRIDE*m + phi], flattened to
    [B, H, STRIDE*NM]. This makes each kw tap's matmul moving operand a
    contiguous SBUF slice. Split per core."""
    B, H, W = x.shape
    NM = W // STRIDE + 1
    xb = np.zeros((B, H, STRIDE * NM), _BF16)
    xb[:, :, :W] = x.astype(_BF16)
    xd = np.ascontiguousarray(
        xb.reshape(B, H, NM, STRIDE).transpose(0, 1, 3, 2)
    ).reshape(B, H, STRIDE * NM)
    return [
        np.ascontiguousarray(xd[c * n_img : (c + 1) * n_img])
        for c in range(N_CORES)
    ]


_CACHE = {}


def _get_nc(n_img, H, W):
    key = (n_img, H, W)
    if key not in _CACHE:
        _CACHE[key] = build_nc(n_img, H, W)
    return _CACHE[key]


def kernel(x, weight):
    x = np.asarray(x, np.float32)
    weight = np.asarray(weight, np.float32)
    B, H, W = x.shape
    assert B % N_CORES == 0
    n_img = B // N_CORES
    nc, starts, nH, nW, n_groups = _get_nc(n_img, H, W)
    wb = build_wband(weight)
    x_shards = prep_x(x, n_img)
    in_maps = [{"x": x_shards[c], "wb": wb} for c in range(N_CORES)]
    results = run_bass_kernel_spmd(
        nc, in_maps, core_ids=list(range(N_CORES))
    ).results
    shards = [
        unscramble(results[c]["out"], starts, nH, nW, n_img)
        for c in range(N_CORES)
    ]
    return np.concatenate(shards, axis=0)


# revision 8
# speedup vs baseline: 895.3817x; 1.2487x over previous
"""Trainium2 Bass kernel for strided-conv-as-linear (nn_ConvNd_60851096649851).

Computation (see reference): x [B,1024,1024] f32, weight [16,256] f32.
16x16 windows at stride 8 -> 127x127 patches; per patch y = W @ flat(window)
(16 outputs), reshaped to a 4x4 tile of the [B,508,508] output.

Strategy: data-parallel over batch (4 images per core, 8 cores).
Per image: 9 overlapping 128-row window tiles (stride 120 rows, last tile
anchored at H-128). Rows live on SBUF partitions (natural layout). For each
window tile, out[(i_l,o), j] = sum_kw Wband_kw[row, (i_l,o)]^T @ x[row, 8j+kw]
where Wband_kw is the banded weight (nonzero at row = 8*i_l + kh): 16
accumulating bf16 matmuls with stride-8 column APs, K=128, M=128
(i_l in [0,8) x 16 outputs per half; two halves cover 15 patches/tile).
Three tiles are batched in the moving dim (N = 3*128 = 384) so the PE
streams at 1 cycle/column in bf16. PSUM holds the f32 outputs; DVE
copies to SBUF; contiguous DMA out. Host casts x to bf16 (and pads the
row width by 8 so the padded j column stays in bounds), prepares banded
bf16 weights, and unscrambles the device output layout into [B,508,508].
"""

import os
import sys

sys.path.insert(0, "/opt/trn_rl_repo")
os.environ.setdefault("JAX_PLATFORMS", "cpu")

import numpy as np

import concourse.bass as bass  # noqa: F401
import concourse.tile as tile
from concourse import bacc, mybir
from concourse.bass_utils import run_bass_kernel_spmd

N_CORES = 8
KH = KW = 16
STRIDE = 8
D0 = D1 = 4  # per-patch output tile
OC = 16  # outputs per patch = D0*D1
PATCHES_PER_TILE = 15  # full patches in a 128-row window
GROUP = 3  # window tiles batched per matmul (moving dim)

_MM_DTYPE = mybir.dt.bfloat16
_BF16 = mybir.dt.np(mybir.dt.bfloat16)


def _tile_starts(H):
    """Start rows of 128-row window tiles covering all patch rows."""
    nH = (H - KH) // STRIDE + 1
    starts = []
    i = 0
    while i < nH:
        s = min(STRIDE * i, H - 128)
        starts.append(s)
        if s == H - 128:
            break
        i += PATCHES_PER_TILE
    return starts, nH


def build_wband(weight):
    """Banded weights: [128, KW*2*128] bf16.

    wb[p, kw, h, m] = W[o, kh*16+kw] where p = 8*(i_l+8h)+kh, m = i_l*16+o,
    i_l in [0,8), valid patch slots i_l+8h in [0,15).
    """
    W4 = np.asarray(weight, np.float32).reshape(OC, KH, KW)
    wb = np.zeros((128, KW, 2, 128), np.float32)
    for h in range(2):
        for il in range(8):
            ig = il + 8 * h
            if ig >= PATCHES_PER_TILE:
                continue
            for kh in range(KH):
                p = 8 * ig + kh
                wb[p, :, h, il * OC : (il + 1) * OC] = W4[:, kh, :].T
    return np.ascontiguousarray(wb.reshape(128, KW * 2 * 128)).astype(_BF16)


def build_nc(n_img, H, W):
    """Build the per-core Bass program. Returns compiled nc."""
    starts, nH = _tile_starts(H)
    nW = (W - KW) // STRIDE + 1
    n_tiles = len(starts)
    assert n_tiles % GROUP == 0, (n_tiles, GROUP)
    n_groups = n_tiles // GROUP
    nWp = ((nW + 3) // 4) * 4  # pad j to multiple of 4
    NF = GROUP * nWp  # moving free size per matmul
    NM = W // STRIDE + 1  # phase-deinterleaved positions per row (129)
    WS = STRIDE * NM  # host-padded row width (phase-major layout)

    nc = bacc.Bacc(
        "TRN2", target_bir_lowering=False, debug=False, num_devices=N_CORES
    )
    f32 = mybir.dt.float32
    x_d = nc.dram_tensor("x", [n_img, H, WS], _MM_DTYPE, kind="ExternalInput").ap()
    wb_d = nc.dram_tensor(
        "wb", [128, KW * 2 * 128], _MM_DTYPE, kind="ExternalInput"
    ).ap()
    out_d = nc.dram_tensor(
        "out", [n_img, n_groups, 2, 128, NF], f32, kind="ExternalOutput"
    ).ap()

    with tile.TileContext(nc) as tc:
        with (
            tc.tile_pool(name="wbp", bufs=1) as wbp,
            tc.tile_pool(name="xp", bufs=6) as xp,
            tc.tile_pool(name="psp", bufs=6, space="PSUM") as psp,
            tc.tile_pool(name="op", bufs=4) as op,
        ):
            wb_sb = wbp.tile([128, KW * 2 * 128], _MM_DTYPE)
            nc.sync.dma_start(wb_sb[:], wb_d[:])

            for b in range(n_img):
                for g in range(n_groups):
                    xg = xp.tile(
                        [128, GROUP * WS], _MM_DTYPE, name=f"xg{b}_{g}", tag="xg"
                    )
                    for t in range(GROUP):
                        s = starts[g * GROUP + t]
                        nc.gpsimd.dma_start(
                            xg[:, t * WS : (t + 1) * WS], x_d[b, s : s + 128, :]
                        )
                    xg4 = xg.rearrange(
                        "p (t f m) -> p t f m", t=GROUP, f=STRIDE, m=NM
                    )
                    ps = [
                        psp.tile([128, NF], f32, name=f"ps_{b}_{g}_{h}", tag="ps")
                        for h in range(2)
                    ]
                    for kw in range(KW):
                        m0 = 0 if kw < STRIDE else 1
                        rhs = xg4[:, :, kw % STRIDE, m0 : m0 + nWp]
                        for h in range(2):
                            lhsT = wb_sb[
                                :, (kw * 2 + h) * 128 : (kw * 2 + h) * 128 + 128
                            ]
                            nc.tensor.matmul(
                                ps[h][:],
                                lhsT,
                                rhs,
                                start=(kw == 0),
                                stop=(kw == KW - 1),
                            )
                    for h in range(2):
                        ob = op.tile([128, NF], f32, name="ob")
                        nc.vector.tensor_copy(ob[:], ps[h][:])
                        nc.sync.dma_start(out_d[b, g, h], ob[:])
    nc.compile()
    return nc, starts, nH, nW, n_groups


def unscramble(dev_out, starts, nH, nW, n_img):
    """dev_out [n_img, n_groups, 2, 128, GROUP*nWp] -> [n_img, nH*4, nW*4]."""
    n_groups = dev_out.shape[1]
    nWp = ((nW + 3) // 4) * 4
    dev = dev_out.reshape(n_img, n_groups, 2, 8, D0, D1, GROUP, nWp)[..., :nW]
    out5 = np.empty((n_img, nH, D0, nW, D1), np.float32)
    filled = np.zeros(nH, bool)
    for g in range(n_groups):
        for t in range(GROUP):
            tau = g * GROUP + t
            i0 = starts[tau] // STRIDE
            for h in range(2):
                for il in range(8):
                    ig = il + 8 * h
                    i = i0 + ig
                    if ig >= PATCHES_PER_TILE or i >= nH or filled[i]:
                        continue
                    # dev[b, g, h, il, d0, d1, t, j] -> out5[b, i, d0, j, d1]
                    out5[:, i] = dev[:, g, h, il, :, :, t, :].transpose(0, 1, 3, 2)
                    filled[i] = True
    assert filled.all()
    return out5.reshape(n_img, nH * D0, nW * D1)


def prep_x(x, n_img):
    """Cast to bf16, pad width by STRIDE, and phase-deinterleave each row:
    xd[b, r, phi, m] = x_pad[b, r, STRIDE*m + phi], flattened to
    [B, H, STRIDE*NM]. This makes each kw tap's matmul moving operand a
    contiguous SBUF slice. Split per core."""
    B, H, W = x.shape
    NM = W // STRIDE + 1
    xb = np.zeros((B, H, STRIDE * NM), _BF16)
    xb[:, :, :W] = x.astype(_BF16)
    xd = np.ascontiguousarray(
        xb.reshape(B, H, NM, STRIDE).transpose(0, 1, 3, 2)
    ).reshape(B, H, STRIDE * NM)
    return [
        np.ascontiguousarray(xd[c * n_img : (c + 1) * n_img])
        for c in range(N_CORES)
    ]


_CACHE = {}


def _get_nc(n_img, H, W):
    key = (n_img, H, W)
    if key not in _CACHE:
        _CACHE[key] = build_nc(n_img, H, W)
    return _CACHE[key]


def kernel(x, weight):
    x = np.asarray(x, np.float32)
    weight = np.asarray(weight, np.float32)
    B, H, W = x.shape
    assert B % N_CORES == 0
    n_img = B // N_CORES
    nc, starts, nH, nW, n_groups = _get_nc(n_img, H, W)
    wb = build_wband(weight)
    x_shards = prep_x(x, n_img)
    in_maps = [{"x": x_shards[c], "wb": wb} for c in range(N_CORES)]
    results = run_bass_kernel_spmd(
        nc, in_maps, core_ids=list(range(N_CORES))
    ).results
    shards = [
        unscramble(results[c]["out"], starts, nH, nW, n_img)
        for c in range(N_CORES)
    ]
    return np.concatenate(shards, axis=0)
